# revision 30
# baseline (speedup 1.0000x reference)
"""Trainium2 Bass kernel for nn_CombinedGraphLayer (LSH-binned GHConv message passing).

Contract: kernel(**inputs) takes FULL inputs (x [16,12800,256], msk [16,12800],
training scalar + weights), returns FULL output [16,12800,256].

Strategy: pure data-parallel over batch (8 cores). The wall clock is dominated
by host<->device transfer through the tunnel (~50-70 MB/s each way, full
duplex), so the I/O is compressed and pipelined:
  - x is shipped as int16 (one global scale). LayerNorm is scale/shift
    invariant per row, so the device consumes the raw int16 values directly;
    only the LN epsilon is rescaled (shipped as a tiny input).
  - the output is shipped as int8 with a per-row fp32 scale packed into 4
    extra int8 columns ([12800, 260] per batch), dequantized on host.
  - the device program processes ONE batch per core; the 16-batch problem is
    two pipelined calls, so chunk 1's upload overlaps chunk 0's download.

Device pipeline per batch (unchanged from the fp32 version except I/O):
  phase A  (per 128-point chunk): layernorm -> ffn_dist -> LSH argmax key;
           accumulate per-chunk key histograms; keep z*m / x_dist resident.
  phase A2: counting-sort ranks via prefix sums; indirect-scatter packed rows
           into sorted order (DRAM).
  phase B  (per 128-point bin): pairwise gaussian adjacency + 2 GHConv
           layers; quantize to int8 + row scale; indirect-scatter to the
           original row order.
"""

import threading

import numpy as np

import concourse.bass as bass
import concourse.tile as tile
from concourse import mybir
from concourse.masks import make_identity

dt = mybir.dt
OP = mybir.AluOpType
AF = mybir.ActivationFunctionType
IOA = bass.IndirectOffsetOnAxis

F = 256       # feature dim
D = 128       # distance dim
BIN = 128
OW = F + 4    # int8 output row: 256 data + 4 bytes fp32 row scale

# packed row layout (fp32): [ zm(0:256) | xd(256:384) | m(384) | idx(385) | pad ]
RW = 388
COL_M = 384
COL_IDX = 385


def split_excess_waits(nc):
    """This walrus build rejects instructions carrying more than a couple of
    sem waits (1 for CTRL-class like Drain, ~2 for compute). Move excess
    waits onto extra Drains inserted just before, on the same engine."""
    for f in nc.m.functions:
        for b in f.blocks:
            new_insts = []
            for inst in b.instructions:
                si = getattr(inst, "sync_info", None)
                ow = list(si.on_wait) if si is not None and si.on_wait else []
                limit = 1
                if len(ow) > limit and inst.engine is not None:
                    keep = ow[-limit:]
                    for w in ow[:-limit]:
                        d = mybir.InstNoOp(
                            name=nc.get_next_instruction_name(), ins=[], outs=[]
                        )
                        d.engine = inst.engine
                        d.sync_info = mybir.SyncInfo(on_wait=[w], on_update=[])
                        new_insts.append(d)
                    si.on_wait = keep
                new_insts.append(inst)
            b.instructions = new_insts


def build(nch, KB):
    """Build the Bass module: ONE batch of nch 128-point chunks per core.
    KB = compact-output row bound: unmasked rows scatter to [0, cnt) in
    original order, masked rows (output exactly 0) to dump rows [KB, KB+128)."""
    NP = nch * BIN            # points per batch
    NBINS = nch               # bins == chunks
    CB = NBINS // 2           # used codebook columns
    NK = 2 * NBINS - 1        # distinct sort keys: 0 .. NK-1
    NKP = NK + 1              # padded width
    f32 = dt.float32
    bf16 = dt.bfloat16

    nc = bass.Bass("TRN2", target_bir_lowering=False, debug=False)

    x_in = nc.dram_tensor("x", [NP, F], dt.int8, kind="ExternalInput").ap()
    m_in = nc.dram_tensor("m", [NP, 1], f32, kind="ExternalInput").ap()
    key_in = nc.dram_tensor("keyf", [NP, 1], f32, kind="ExternalInput").ap()
    eps_in = nc.dram_tensor("epsr", [NP, 1], f32, kind="ExternalInput").ap()
    wspec = [
        ("W1g", [F, D]), ("b1gb", [1, D]), ("W2", [D, D]), ("b2", [1, D]),
        ("th0", [F, F]), ("Wh0", [F, F]), ("Wt0", [F, F]),
        ("bth0", [1, F]), ("bhh0", [1, F]), ("bgt0", [1, F]),
        ("th1", [F, F]), ("Wh1", [F, F]), ("Wt1", [F, F]), ("bt1", [1, F]),
    ]
    wdram = {n: nc.dram_tensor(n, s, f32, kind="ExternalInput").ap() for n, s in wspec}
    outq_d = nc.dram_tensor("outq0", [KB + 128, OW], dt.int8,
                            kind="ExternalOutput").ap()
    psort_d = nc.dram_tensor("psort0", [NP, RW], f32, kind="Internal").ap()

    with tile.TileContext(nc) as tc:
        with tc.tile_pool(name="init", bufs=1) as ip:
            ident = ip.tile([128, 128], f32)
            make_identity(nc, ident[:])
            ones_row_f = ip.tile([1, 128], f32)
            nc.vector.memset(ones_row_f[:], 1.0)
            eps_fx = ip.tile([128, 1], f32)
            nc.vector.memset(eps_fx[:], 1e-6)
            iota_p_i = ip.tile([128, 1], dt.int32)
            nc.gpsimd.iota(iota_p_i[:], [[0, 1]], base=0, channel_multiplier=1)
            iota_p_f = ip.tile([128, 1], f32)
            nc.vector.tensor_copy(iota_p_f[:], iota_p_i[:])
            dump_col = ip.tile([128, 1], f32)
            nc.vector.tensor_scalar_add(dump_col[:], iota_p_f[:], float(KB))
            iota_row_i = ip.tile([128, NKP], dt.int32)
            nc.gpsimd.iota(iota_row_i[:], [[1, NKP]], base=0, channel_multiplier=0)
            iota_row_f = ip.tile([128, NKP], f32)
            nc.vector.tensor_copy(iota_row_f[:], iota_row_i[:])
            # strictly-lower mask Tp[p', p] = 1 if p' < p  (for within-chunk cumsum)
            iota_r128_i = ip.tile([128, 128], dt.int32)
            nc.gpsimd.iota(iota_r128_i[:], [[1, 128]], base=0, channel_multiplier=0)
            iota_r128_f = ip.tile([128, 128], f32)
            nc.vector.tensor_copy(iota_r128_f[:], iota_r128_i[:])
            Tp_bf = ip.tile([128, 128], bf16)
            nc.vector.tensor_scalar(
                out=Tp_bf[:], in0=iota_r128_f[:], scalar1=iota_p_f[:],
                scalar2=None, op0=OP.is_gt)
            ones_col_bf = ip.tile([128, 1], bf16)
            nc.vector.memset(ones_col_bf[:], 1.0)

            # weights to SBUF
            wsb = {}
            for n, s in wspec:
                shp = ([128, s[0] // 128, s[1]] if s[0] > 128 else list(s))
                src = (wdram[n].rearrange("(c p) m -> p c m", p=128)
                       if s[0] > 128 else wdram[n][:])
                t = ip.tile(shp, f32, tag=f"w_{n}")
                nc.gpsimd.dma_start(out=t[:], in_=src)
                wsb[n] = t

            _one_batch(tc, nc, nch, NP, NBINS, CB, NK, NKP,
                       x_in, m_in, key_in, eps_in, wsb, outq_d, psort_d,
                       ident, eps_fx, iota_p_f, iota_row_f, Tp_bf,
                       ones_col_bf, ones_row_f, dump_col)

    split_excess_waits(nc)
    return nc


def _one_batch(tc, nc, nch, NP, NBINS, CB, NK, NKP,
               x_in, m_in, key_in, eps_in, wsb, outq_d, psort_d,
               ident, eps_fx, iota_p_f, iota_row_f, Tp_bf,
               ones_col_bf, ones_row_f, dump_col):
    f32 = dt.float32
    bf16 = dt.bfloat16

    with tc.tile_pool(name="res", bufs=1) as rp, \
         tc.tile_pool(name="resps", bufs=1, space="PSUM") as rpp:
        packed = rp.tile([128, nch, RW], f32)     # resident z*m / xd / m / cidx
        key_all = rp.tile([128, nch], f32)
        rank_f = rp.tile([128, nch], f32)
        rank_u = rp.tile([128, nch], dt.uint32)
        T_lo = rpp.tile([NBINS, nch], f32, space="PSUM")
        T_hi = rpp.tile([NBINS - 1, nch], f32, space="PSUM")
        cnt_ps = rpp.tile([1, nch], f32, space="PSUM")

        # ---------------- phase A ----------------
        with tc.tile_pool(name="pa", bufs=3) as pa, \
             tc.tile_pool(name="paps", bufs=1, space="PSUM") as pap:
            for c in range(nch):
                row0 = c * 128
                x_t8 = pa.tile([128, F], dt.int8)
                nc.sync.dma_start(out=x_t8[:], in_=x_in[row0:row0 + 128, :])
                x_t = pa.tile([128, F], f32)
                nc.vector.tensor_copy(x_t[:], x_t8[:])
                nc.sync.dma_start(out=packed[:, c, COL_M:COL_M + 1],
                                  in_=m_in[row0:row0 + 128, :])
                m_ap = packed[:, c, COL_M:COL_M + 1]
                eps_t = pa.tile([128, 1], f32)
                nc.sync.dma_start(out=eps_t[:], in_=eps_in[row0:row0 + 128, :])
                # within-chunk exclusive cumsum of the mask + per-chunk count
                # (for the compact-output row index)
                mcol_bf = pa.tile([128, 1], bf16)
                nc.vector.tensor_copy(mcol_bf[:], m_ap)
                cw_ps = pap.tile([128, 1], f32, space="PSUM")
                nc.tensor.matmul(cw_ps[:], lhsT=Tp_bf[:], rhs=mcol_bf[:],
                                 start=True, stop=True)
                nc.scalar.activation(out=packed[:, c, COL_IDX:COL_IDX + 1],
                                     in_=cw_ps[:], func=AF.Copy)
                nc.tensor.matmul(cnt_ps[0:1, c:c + 1], lhsT=mcol_bf[:],
                                 rhs=ones_col_bf[:], start=True, stop=True)

                st = pa.tile([128, 6], f32)
                nc.vector.bn_stats(out=st[:], in_=x_t[:])
                mv = pa.tile([128, 2], f32)
                nc.vector.bn_aggr(out=mv[:], in_=st[:])
                nc.scalar.activation(out=mv[:, 1:2], in_=mv[:, 1:2],
                                     func=AF.Sqrt, bias=eps_t[:])
                nc.vector.reciprocal(out=mv[:, 1:2], in_=mv[:, 1:2])
                z_t = pa.tile([128, F], f32)
                nc.vector.tensor_scalar(
                    out=z_t[:], in0=x_t[:], scalar1=mv[:, 0:1],
                    scalar2=mv[:, 1:2], op0=OP.subtract, op1=OP.mult)
                # zm into packed (gpsimd: SBUF only)
                nc.gpsimd.tensor_scalar_mul(packed[:, c, 0:F], z_t[:], m_ap)

                # zT (feature-major) for the ffn matmuls
                zT_ps = pap.tile([128, 2, 128], f32, space="PSUM")
                for k in range(2):
                    nc.tensor.transpose(zT_ps[:, k, :],
                                        z_t[:, k * 128:(k + 1) * 128], ident[:])
                zT_sb = pa.tile([128, 2, 128], f32)
                nc.scalar.activation(out=zT_sb[:], in_=zT_ps[:], func=AF.Copy)

                # hT = W1g^T zT + b1gb  (feature-major [D, pts])
                h_ps = pap.tile([128, 128], f32, space="PSUM")
                nc.tensor.matmul(h_ps[:], lhsT=wsb["W1g"][:, 0, :],
                                 rhs=zT_sb[:, 0, :], start=True, stop=False)
                nc.tensor.matmul(h_ps[:], lhsT=wsb["W1g"][:, 1, :],
                                 rhs=zT_sb[:, 1, :], start=False, stop=False)
                nc.tensor.matmul(h_ps[:], lhsT=wsb["b1gb"][:],
                                 rhs=ones_row_f[:], start=False, stop=True)
                # elu
                e_t = pa.tile([128, 128], f32)
                nc.vector.tensor_scalar_min(e_t[:], h_ps[:], 0.0)
                nc.scalar.activation(out=e_t[:], in_=e_t[:], func=AF.Exp)
                r_t = pa.tile([128, 128], f32)
                nc.scalar.activation(out=r_t[:], in_=h_ps[:], func=AF.Relu)
                hTe = pa.tile([128, 128], f32)
                nc.vector.scalar_tensor_tensor(
                    out=hTe[:], in0=e_t[:], scalar=-1.0, in1=r_t[:],
                    op0=OP.add, op1=OP.add)

                # xdT = W2^T hTe + b2
                xdT_ps = pap.tile([128, 128], f32, space="PSUM")
                nc.tensor.matmul(xdT_ps[:], lhsT=wsb["W2"][:], rhs=hTe[:],
                                 start=True, stop=False)
                nc.tensor.matmul(xdT_ps[:], lhsT=wsb["b2"][:],
                                 rhs=ones_row_f[:], start=False, stop=True)
                xdT_sb = pa.tile([128, 128], f32)
                nc.scalar.activation(out=xdT_sb[:], in_=xdT_ps[:], func=AF.Copy)
                # xd point-major into packed
                xd_ps = pap.tile([128, 128], f32, space="PSUM")
                nc.tensor.transpose(xd_ps[:], xdT_sb[:], ident[:])
                nc.vector.tensor_copy(packed[:, c, F:F + 128], xd_ps[:])

                # LSH key: host-computed (bit-exact vs the reference argmax)
                nc.sync.dma_start(out=key_all[:, c:c + 1],
                                  in_=key_in[row0:row0 + 128, :])
                # onehot -> per-chunk histogram columns
                oh = pa.tile([128, NKP], bf16)
                nc.vector.tensor_scalar(
                    out=oh[:], in0=iota_row_f[:, 0:NKP], scalar1=key_all[:, c:c + 1],
                    scalar2=None, op0=OP.is_equal)
                nc.tensor.matmul(T_lo[:, c:c + 1], lhsT=oh[:, 0:NBINS],
                                 rhs=ones_col_bf[:], start=True, stop=True)
                nc.tensor.matmul(T_hi[:, c:c + 1], lhsT=oh[:, NBINS:NK],
                                 rhs=ones_col_bf[:], start=True, stop=True)

        # ---------------- phase A2: ranks + sort scatter ----------------
        with tc.tile_pool(name="pa2", bufs=2) as p2, \
             tc.tile_pool(name="pa2ps", bufs=1, space="PSUM") as p2p:
            # compact-row offsets: exclusive scan of per-chunk mask counts,
            # broadcast to all partitions, added to the within-chunk ranks
            cnt_sb = p2.tile([1, nch], f32)
            nc.scalar.activation(out=cnt_sb[:], in_=cnt_ps[0:1, :], func=AF.Copy)
            cnt_inc = p2.tile([1, nch], f32)
            nc.vector.tensor_tensor_scan(cnt_inc[:], cnt_sb[:], cnt_sb[:], 0.0,
                                         OP.add, OP.bypass)
            cnt_exc = p2.tile([1, nch], f32)
            nc.vector.tensor_sub(cnt_exc[:], cnt_inc[:], cnt_sb[:])
            offs_bc = p2p.tile([128, nch], f32, space="PSUM")
            nc.tensor.matmul(offs_bc[:], lhsT=ones_row_f[:], rhs=cnt_exc[:],
                             start=True, stop=True)
            nc.vector.tensor_tensor(
                out=packed[:, :, COL_IDX], in0=packed[:, :, COL_IDX],
                in1=offs_bc[:], op=OP.add)
            Tl_sb = p2.tile([NBINS, nch], f32)
            nc.scalar.activation(out=Tl_sb[:], in_=T_lo[:], func=AF.Copy)
            Th_sb = p2.tile([NBINS - 1, nch], f32)
            nc.scalar.activation(out=Th_sb[:], in_=T_hi[:], func=AF.Copy)
            # inclusive scan along chunks
            Sl_in = p2.tile([NBINS, nch], f32)
            nc.vector.tensor_tensor_scan(Sl_in[:], Tl_sb[:], Tl_sb[:], 0.0,
                                         OP.add, OP.bypass)
            Sh_in = p2.tile([NBINS - 1, nch], f32)
            nc.vector.tensor_tensor_scan(Sh_in[:], Th_sb[:], Th_sb[:], 0.0,
                                         OP.add, OP.bypass)
            # exclusive
            Sl_ex = p2.tile([NBINS, nch], f32)
            nc.vector.tensor_sub(Sl_ex[:], Sl_in[:], Tl_sb[:])
            Sh_ex = p2.tile([NBINS - 1, nch], f32)
            nc.vector.tensor_sub(Sh_ex[:], Sh_in[:], Th_sb[:])
            # grand totals -> key-offsets (exclusive cumsum over keys)
            grow_ps = p2p.tile([1, NKP], f32, space="PSUM")
            nc.tensor.transpose(grow_ps[:, 0:NBINS], Sl_in[:, nch - 1:nch],
                                ident[0:NBINS, 0:NBINS])
            nc.tensor.transpose(grow_ps[:, NBINS:NK], Sh_in[:, nch - 1:nch],
                                ident[0:NBINS - 1, 0:NBINS - 1])
            grow_sb = p2.tile([1, NKP], f32)
            nc.vector.memset(grow_sb[:], 0.0)
            nc.scalar.activation(out=grow_sb[:, 0:NK], in_=grow_ps[:, 0:NK],
                                 func=AF.Copy)
            ginc = p2.tile([1, NKP], f32)
            nc.vector.tensor_tensor_scan(ginc[:], grow_sb[:], grow_sb[:], 0.0,
                                         OP.add, OP.bypass)
            gexc = p2.tile([1, NKP], f32)
            nc.vector.tensor_sub(gexc[:], ginc[:], grow_sb[:])
            offs_ps = p2p.tile([NBINS, 2], f32, space="PSUM")
            nc.tensor.transpose(offs_ps[:, 0:1], gexc[:, 0:NBINS], ident[0:1, 0:1])
            nc.tensor.transpose(offs_ps[0:NBINS - 1, 1:2], gexc[:, NBINS:NK],
                                ident[0:1, 0:1])
            offs_sb = p2.tile([NBINS, 2], f32)
            nc.scalar.activation(out=offs_sb[:], in_=offs_ps[:], func=AF.Copy)
            nc.vector.tensor_scalar_add(Sl_ex[:], Sl_ex[:], offs_sb[:, 0:1])
            nc.vector.tensor_scalar_add(Sh_ex[:], Sh_ex[:],
                                        offs_sb[0:NBINS - 1, 1:2])
            # St[c, k] = base for chunk c / key k
            St_ps = p2p.tile([nch, NKP], f32, space="PSUM")
            nc.tensor.transpose(St_ps[:, 0:NBINS], Sl_ex[:],
                                ident[0:NBINS, 0:NBINS])
            nc.tensor.transpose(St_ps[:, NBINS:NK], Sh_ex[:],
                                ident[0:NBINS - 1, 0:NBINS - 1])
            St_sb = p2.tile([nch, NKP], f32)
            nc.vector.memset(St_sb[:], 0.0)
            nc.scalar.activation(out=St_sb[:, 0:NK], in_=St_ps[:, 0:NK],
                                 func=AF.Copy)

            for c in range(nch):
                oh2 = p2.tile([128, NKP], bf16)
                nc.vector.tensor_scalar(
                    out=oh2[:], in0=iota_row_f[:, 0:NKP],
                    scalar1=key_all[:, c:c + 1], scalar2=None, op0=OP.is_equal)
                St_row = p2.tile([1, NKP], f32)
                nc.sync.dma_start(out=St_row[:], in_=St_sb[c:c + 1, :])
                C_ps = p2p.tile([128, NKP], f32, space="PSUM")
                nc.tensor.matmul(C_ps[:], lhsT=Tp_bf[:], rhs=oh2[:],
                                 start=True, stop=False)
                nc.tensor.matmul(C_ps[:], lhsT=ones_row_f[:],
                                 rhs=St_row[:], start=False, stop=True)
                scr = p2.tile([128, NKP], f32)
                nc.vector.tensor_tensor(out=scr[:], in0=oh2[:], in1=C_ps[:],
                                        op=OP.mult)
                nc.vector.tensor_reduce(
                    out=rank_f[:, c:c + 1], in_=scr[:],
                    axis=mybir.AxisListType.X, op=OP.add)
            nc.vector.tensor_copy(rank_u[:], rank_f[:])
            for c in range(nch):
                nc.gpsimd.indirect_dma_start(
                    out=psort_d[:],
                    out_offset=IOA(ap=rank_u[:, c:c + 1], axis=0),
                    in_=packed[:, c, :], in_offset=None)

    # ---------------- phase B: adjacency + GHConv per bin ----------------
    with tc.tile_pool(name="pb", bufs=4) as pb, \
         tc.tile_pool(name="pbps", bufs=1, space="PSUM") as pbp:
        for s in range(NBINS):
            pk = pb.tile([128, RW], f32)
            nc.sync.dma_start(out=pk[:], in_=psort_d[s * 128:(s + 1) * 128, :])
            m_ap = pk[:, COL_M:COL_M + 1]
            # V cols: [na, one, one, na, m]; transposed pair/row tiles all
            # land at partition base 0 (matmul requires equal bases).
            V = pb.tile([128, 5], f32)
            sq = pb.tile([128, 128], f32)
            nc.scalar.activation(out=sq[:], in_=pk[:, F:F + 128],
                                 func=AF.Square, accum_out=V[:, 0:1])
            nc.gpsimd.memset(V[:, 1:3], 1.0)
            nc.gpsimd.tensor_copy(V[:, 3:4], V[:, 0:1])
            nc.gpsimd.tensor_copy(V[:, 4:5], m_ap)
            vt_ps = pbp.tile([2, 384], f32, space="PSUM")
            nc.tensor.transpose(vt_ps[0:2, 0:128], V[:, 0:2], ident[:])
            VTa = pb.tile([2, 128], f32)
            nc.scalar.activation(out=VTa[:], in_=vt_ps[0:2, 0:128],
                                 func=AF.Copy)
            nc.tensor.transpose(vt_ps[0:2, 128:256], V[:, 2:4], ident[:])
            VTb = pb.tile([2, 128], f32)
            nc.scalar.activation(out=VTb[:], in_=vt_ps[0:2, 128:256],
                                 func=AF.Copy)
            nc.tensor.transpose(vt_ps[0:1, 256:384], V[:, 4:5], ident[:])
            mT_sb = pb.tile([1, 128], f32)
            nc.scalar.activation(out=mT_sb[:], in_=vt_ps[0:1, 256:384],
                                 func=AF.Copy)
            # d2 = na_i - 2 xd xd^T + na_j ; M2 = m_i m_j
            adj_ps = pbp.tile([128, 384], f32, space="PSUM")
            xdT_ps = adj_ps[:, 0:128]
            d2_ps = adj_ps[:, 128:256]
            M2_ps = adj_ps[:, 256:384]
            nc.tensor.transpose(xdT_ps, pk[:, F:F + 128], ident[:])
            xdT = pb.tile([128, 128], f32)
            nc.scalar.activation(out=xdT[:], in_=xdT_ps, func=AF.Copy)
            xdTm2 = pb.tile([128, 128], f32)
            nc.scalar.activation(out=xdTm2[:], in_=xdT_ps, func=AF.Copy,
                                 scale=-2.0)
            nc.tensor.matmul(d2_ps, lhsT=xdTm2[:], rhs=xdT[:],
                             start=True, stop=False)
            nc.tensor.matmul(d2_ps, lhsT=VTa[:], rhs=VTb[:],
                             start=False, stop=True)
            nc.tensor.matmul(M2_ps, lhsT=mT_sb[:], rhs=mT_sb[:],
                             start=True, stop=True)
            dsc = pb.tile([128, 128], f32)
            nc.vector.tensor_scalar_max(dsc[:], d2_ps[:], 1e-6)
            nc.scalar.activation(out=dsc[:], in_=dsc[:], func=AF.Sqrt)
            nc.scalar.activation(out=dsc[:], in_=dsc[:], func=AF.Exp,
                                 scale=-0.1)
            dm = pb.tile([128, 128], f32)
            ind = pb.tile([128, 1], f32)
            nc.vector.scalar_tensor_tensor(
                out=dm[:], in0=dsc[:], scalar=1.0, in1=M2_ps[:],
                op0=OP.mult, op1=OP.mult, accum_out=ind[:])
            nrm = pb.tile([128, 1], f32)
            nc.scalar.activation(out=nrm[:], in_=ind[:], func=AF.Sqrt,
                                 bias=eps_fx[:])
            nc.vector.reciprocal(nrm[:], nrm[:])
            nc.vector.tensor_mul(nrm[:], nrm[:], m_ap)

            xb_ap = pk[:, 0:F]
            for li in range(2):
                sfx = "0" if li == 0 else "1"
                mm1 = pbp.tile([128, 512], f32, space="PSUM")
                mm2 = pbp.tile([128, 512], f32, space="PSUM")
                gat_ps = pbp.tile([128, F], f32, space="PSUM")
                xmT_ps = mm1[:, 0:256]
                hom2_ps = mm1[:, 256:512]
                hom_ps = mm2[:, 0:256]
                het_ps = mm2[:, 256:512]
                for k in range(2):
                    nc.tensor.transpose(
                        xmT_ps.rearrange("p (c q) -> p c q", q=128)[:, k, :],
                        xb_ap[:, k * 128:(k + 1) * 128], ident[:])
                xmT = pb.tile([128, 2, 128], f32)
                nc.scalar.activation(out=xmT[:], in_=xmT_ps, func=AF.Copy)
                mT = mT_sb[:]
                # keep each PSUM accumulation group's matmuls consecutive
                for dst, wn, bias in (
                    (hom_ps, "th" + sfx, "bth0" if li == 0 else None),
                    (het_ps, "Wh" + sfx, "bhh0" if li == 0 else None),
                    (gat_ps[:], "Wt" + sfx,
                     "bgt0" if li == 0 else "bt1"),
                ):
                    for k in range(2):
                        nc.tensor.matmul(
                            dst, lhsT=xmT[:, k, :], rhs=wsb[wn][:, k, :],
                            start=(k == 0), stop=(k == 1 and bias is None))
                    if bias is not None:
                        blhs = mT if li == 0 else ones_row_f[:]
                        nc.tensor.matmul(dst, lhsT=blhs, rhs=wsb[bias][:],
                                         start=False, stop=True)
                fh1 = pb.tile([128, F], f32)
                nc.vector.tensor_scalar_mul(fh1[:], hom_ps[:], nrm[:])
                nc.tensor.matmul(hom2_ps[:], lhsT=dm[:], rhs=fh1[:],
                                 start=True, stop=True)
                gate = pb.tile([128, F], f32)
                nc.scalar.activation(out=gate[:], in_=gat_ps[:], func=AF.Sigmoid)
                fh2 = pb.tile([128, F], f32)
                nc.vector.tensor_scalar_mul(fh2[:], hom2_ps[:], nrm[:])
                nc.vector.tensor_sub(fh2[:], fh2[:], het_ps[:])
                nc.vector.tensor_mul(gate[:], gate[:], fh2[:])
                nc.vector.tensor_add(fh2[:], gate[:], het_ps[:])  # pre-act
                emin = pb.tile([128, F], f32)
                nc.gpsimd.tensor_scalar_min(emin[:], fh2[:], 0.0)
                nc.scalar.activation(out=emin[:], in_=emin[:], func=AF.Exp)
                er = pb.tile([128, F], f32)
                nc.scalar.activation(out=er[:], in_=fh2[:], func=AF.Relu)
                nc.vector.scalar_tensor_tensor(
                    out=emin[:], in0=emin[:], scalar=-1.0, in1=er[:],
                    op0=OP.add, op1=OP.add)
                out_t = pb.tile([128, F], f32)
                nc.gpsimd.tensor_scalar_mul(out_t[:], emin[:], m_ap)
                xb_ap = out_t[:]

            # int8 quantization with per-row scale, packed into one row
            absf = pb.tile([128, F], f32)
            nc.scalar.activation(out=absf[:], in_=xb_ap, func=AF.Abs)
            rmax = pb.tile([128, 1], f32)
            nc.vector.tensor_reduce(out=rmax[:], in_=absf[:],
                                    axis=mybir.AxisListType.X, op=OP.max)
            nc.vector.tensor_scalar_max(rmax[:], rmax[:], 1e-30)
            inv = pb.tile([128, 1], f32)
            nc.vector.reciprocal(inv[:], rmax[:])
            scrow = pb.tile([128, 1], f32)
            nc.vector.tensor_scalar_mul(scrow[:], rmax[:], 1.0 / 127.0)
            qf = pb.tile([128, F], f32)
            nc.vector.tensor_scalar_mul(qf[:], xb_ap, inv[:])
            nc.vector.tensor_scalar(out=qf[:], in0=qf[:], scalar1=127.0,
                                    scalar2=127.0, op0=OP.mult, op1=OP.min)
            q8 = pb.tile([128, OW], dt.int8)
            nc.vector.tensor_copy(q8[:, 0:F], qf[:])
            nc.vector.tensor_copy(q8[:, F:OW], scrow[:].bitcast(dt.int8))

            cidf = pb.tile([128, 1], f32)
            nc.vector.tensor_sub(cidf[:], pk[:, COL_IDX:COL_IDX + 1],
                                 dump_col[:])
            nc.vector.tensor_scalar_mul(cidf[:], cidf[:], m_ap)
            nc.vector.tensor_add(cidf[:], cidf[:], dump_col[:])
            idx_u = pb.tile([128, 1], dt.uint32)
            nc.vector.tensor_copy(idx_u[:], cidf[:])
            nc.gpsimd.indirect_dma_start(
                out=outq_d[:], out_offset=IOA(ap=idx_u[:, 0:1], axis=0),
                in_=q8[:], in_offset=None)


def _fold_weights(inputs):
    g = inputs["ln_gamma"].astype(np.float32)
    be = inputs["ln_beta"].astype(np.float32)
    W1 = inputs["W1"].astype(np.float32)
    b1 = inputs["b1"].astype(np.float32)
    w = {
        "W1g": g[:, None] * W1,
        "b1gb": (b1 + be @ W1)[None, :],
        "W2": inputs["W2"].astype(np.float32),
        "b2": inputs["b2"].astype(np.float32)[None, :],
        "th1": inputs["th1"].astype(np.float32),
        "Wh1": inputs["Wh1"].astype(np.float32),
        "Wt1": inputs["Wt1"].astype(np.float32),
        "bt1": inputs["bt1"].astype(np.float32)[None, :],
    }
    for nm in ("th0", "Wh0", "Wt0"):
        w[nm] = g[:, None] * inputs[nm].astype(np.float32)
    w["bth0"] = (be @ inputs["th0"].astype(np.float32))[None, :]
    w["bhh0"] = (be @ inputs["Wh0"].astype(np.float32))[None, :]
    w["bgt0"] = (inputs["bt0"].astype(np.float32) +
                 be @ inputs["Wt0"].astype(np.float32))[None, :]
    return {k: np.ascontiguousarray(v, dtype=np.float32) for k, v in w.items()}


_BUILD_CACHE = {}


def _get_nc(nch, KB):
    if (nch, KB) not in _BUILD_CACHE:
        _BUILD_CACHE[(nch, KB)] = build(nch, KB)
    return _BUILD_CACHE[(nch, KB)]


_RUNNER_CACHE = {}


def _get_runner(nch, n_cores, KB):
    """Cached jitted SPMD executor (re-jitting per call costs seconds)."""
    key = (nch, n_cores, KB)
    if key in _RUNNER_CACHE:
        return _RUNNER_CACHE[key]
    import jax
    from jax.sharding import Mesh, PartitionSpec, NamedSharding
    from jax.experimental.shard_map import shard_map
    from concourse import bass2jax

    bass2jax.install_neuronx_cc_hook()
    nc = _get_nc(nch, KB)
    partition_name = (nc.partition_id_tensor.name
                      if nc.partition_id_tensor else None)
    in_names, out_names, out_avals, zero_shapes = [], [], [], []
    for alloc in nc.m.functions[0].allocations:
        if not isinstance(alloc, mybir.MemoryLocationSet):
            continue
        name = alloc.memorylocations[0].name
        if alloc.kind == "ExternalInput":
            if name != partition_name:
                in_names.append(name)
        elif alloc.kind == "ExternalOutput":
            out_names.append(name)
            shape = tuple(alloc.tensor_shape)
            dtype = mybir.dt.np(alloc.dtype)
            out_avals.append(jax.core.ShapedArray(shape, dtype))
            zero_shapes.append((shape, dtype))
    n_params = len(in_names)
    all_names = in_names + out_names
    if partition_name is not None:
        all_names = all_names + [partition_name]

    def _body(*args):
        operands = list(args)
        if partition_name is not None:
            operands.append(bass2jax.partition_id_tensor())
        outs = bass2jax._bass_exec_p.bind(
            *operands,
            out_avals=tuple(out_avals),
            in_names=tuple(all_names),
            out_names=tuple(out_names),
            lowering_input_output_aliases=(),
            sim_require_finite=True,
            sim_require_nnan=True,
            nc=nc,
        )
        return tuple(outs)

    devices = jax.devices()[:n_cores]
    mesh = Mesh(np.asarray(devices), ("core",))
    in_specs = (PartitionSpec("core"),) * (n_params + len(out_names))
    out_specs = (PartitionSpec("core"),) * len(out_names)
    sharded = jax.jit(
        shard_map(_body, mesh=mesh, in_specs=in_specs, out_specs=out_specs,
                  check_rep=False),
        keep_unused=True)
    # zero output buffers staged on device ONCE and reused read-only
    shard = NamedSharding(mesh, PartitionSpec("core"))
    dev_zeros = [
        jax.device_put(np.zeros((n_cores * s0[0], *s0[1:]), d), shard)
        for s0, d in zero_shapes]
    runner = (sharded, in_names, out_names, out_avals, dev_zeros, shard)
    _RUNNER_CACHE[key] = runner
    return runner


_WCACHE = {}


def _dev_weights(inputs, n_cores, shard):
    """Folded weights, tiled per-core and staged on device once (cached by
    content hash -- ~5.6MB of wire saved per call)."""
    import hashlib
    import jax

    h = hashlib.blake2b(digest_size=16)
    for kk in ("ln_gamma", "ln_beta", "W1", "b1", "W2", "b2", "th0", "Wh0",
               "Wt0", "bt0", "th1", "Wh1", "Wt1", "bt1"):
        h.update(np.ascontiguousarray(inputs[kk], dtype=np.float32).tobytes())
    dig = h.hexdigest()
    if dig in _WCACHE:
        return _WCACHE[dig]
    w = _fold_weights(inputs)
    gw = {n: jax.device_put(np.concatenate([v] * n_cores, axis=0), shard)
          for n, v in w.items()}
    jax.block_until_ready(list(gw.values()))
    _WCACHE[dig] = gw
    return gw


_JITS = {}


def _get_jits(nbins):
    """Two cached jax-CPU jits:
    keys_of -- the LSH sort keys, with EXACTLY the op sequence of the
      reference (bit-identical argmax; one flipped bin costs ~1e-1 max-rel).
    pack_of -- row-centered int8 quantization of x + the per-row rescaled
      LN epsilon (LayerNorm is invariant to per-row shift/scale)."""
    if nbins in _JITS:
        return _JITS[nbins]
    import jax
    import jax.numpy as jnp

    @jax.jit
    def keys_of(xv, mskv, ln_gamma, ln_beta, W1, b1, W2, b2, codebook):
        mu = jnp.mean(xv, axis=-1, keepdims=True)
        var = jnp.mean(jnp.square(xv - mu), axis=-1, keepdims=True)
        xn = (xv - mu) * jax.lax.rsqrt(var + 1e-6) * ln_gamma + ln_beta
        x_dist = jax.nn.elu(xn @ W1 + b1) @ W2 + b2
        mul = x_dist @ codebook
        cmul = jnp.concatenate([mul, -mul], axis=-1)
        key = jnp.argmax(cmul, axis=-1) + jnp.where(~mskv, nbins - 1, 0)
        return key.astype(jnp.float32)

    @jax.jit
    def pack_of(xv):
        mu = jnp.mean(xv, axis=-1, keepdims=True)
        xc = xv - mu
        rm = jnp.maximum(jnp.max(jnp.abs(xc), axis=-1, keepdims=True), 1e-30)
        c = 127.0 / rm
        q8 = jnp.rint(xc * c).astype(jnp.int8)
        return q8, (1e-6 * c * c).astype(jnp.float32)

    _JITS[nbins] = (keys_of, pack_of)
    return _JITS[nbins]


def run(inputs, nb=2, nch=100, n_cores=8, ghconv_dtype=None, trace=False):
    """inputs: dict with x [B, NP, F] float32, msk [B, NP] bool + weights.
    B must equal n_cores * nb; processed as nb pipelined chunks of one batch
    per core."""
    import jax
    from concurrent.futures import ThreadPoolExecutor

    NP = nch * BIN
    x = np.asarray(inputs["x"])
    msk = np.asarray(inputs["msk"])
    B = x.shape[0]
    assert B == n_cores * nb
    cnts = msk.reshape(B, NP).sum(axis=1).astype(np.int64)
    KB = 56 * BIN                 # 0.56*NP; ~13 sigma above a fair-coin mask
    if int(cnts.max()) > KB:
        KB = NP                   # pathological mask density: no compaction
    sharded, in_names, out_names, out_avals, dev_zeros, shard = _get_runner(
        nch, n_cores, KB)
    keys_of, pack_of = _get_jits(nch)
    gw = _dev_weights(inputs, n_cores, shard)
    kargs = (inputs["ln_gamma"], inputs["ln_beta"], inputs["W1"],
             inputs["b1"], inputs["W2"], inputs["b2"],
             np.ascontiguousarray(inputs["codebook"][:, :nch // 2]))

    out = np.zeros((B, NP, F), np.float32)   # masked rows are exactly 0

    def fetch_chunk(k, fut):
        # runs in a worker: waits for the exec, then streams shards back
        # (overlapping the next chunk's host-side key computation)
        shards = list(fut[0].addressable_shards)
        for s_ in shards:
            s_.data.copy_to_host_async()
        for s_ in shards:
            core = s_.index[0].start // (KB + 128) if s_.index[0].start else 0
            b = k * n_cores + core
            cnt = int(cnts[b])
            a = np.asarray(s_.data)          # [KB+128, 260] int8
            sc = np.ascontiguousarray(a[:cnt, F:OW]).view(np.float32)
            out[b][msk[b]] = a[:cnt, :F].astype(np.float32) * sc

    cpu = jax.devices("cpu")[0]
    with ThreadPoolExecutor(max_workers=4) as ex:
        jobs = []
        with jax.default_device(cpu):
            for k in range(nb):
                xs = x[k * n_cores:(k + 1) * n_cores]
                ms = msk[k * n_cores:(k + 1) * n_cores]
                # pack + start the async x upload FIRST, so the wire streams
                # while the (CPU-bound) exact key computation runs
                q8, epsr = pack_of(xs)
                xq = np.asarray(q8).reshape(n_cores * NP, F)
                dxq = jax.device_put(xq, shard)
                epsr = np.asarray(epsr).reshape(n_cores * NP, 1)
                keys = np.asarray(keys_of(xs, ms, *kargs)).reshape(-1, 1)
                mf = np.ascontiguousarray(
                    ms.reshape(n_cores * NP, 1), dtype=np.float32)
                args = [dxq if n == "x" else mf if n == "m" else
                        keys if n == "keyf" else epsr if n == "epsr" else gw[n]
                        for n in in_names]
                fut = sharded(*args, *dev_zeros)
                jobs.append(ex.submit(fetch_chunk, k, fut))
        for j in jobs:
            j.result()
    return out, None


def kernel(**inputs):
    out, _ = run(inputs, nb=2, nch=100, n_cores=8)
    return out


# revision 31
# speedup vs baseline: 1.1630x; 1.1630x over previous
"""Trainium2 Bass kernel for nn_CombinedGraphLayer (LSH-binned GHConv message passing).

Contract: kernel(**inputs) takes FULL inputs (x [16,12800,256], msk [16,12800],
training scalar + weights), returns FULL output [16,12800,256].

Strategy: pure data-parallel over batch (8 cores). The wall clock is dominated
by host<->device transfer through the tunnel (~50-70 MB/s each way, full
duplex), so the I/O is compressed and pipelined:
  - x is shipped as int16 (one global scale). LayerNorm is scale/shift
    invariant per row, so the device consumes the raw int16 values directly;
    only the LN epsilon is rescaled (shipped as a tiny input).
  - the output is shipped as int8 with a per-row fp32 scale packed into 4
    extra int8 columns ([12800, 260] per batch), dequantized on host.
  - the device program processes ONE batch per core; the 16-batch problem is
    two pipelined calls, so chunk 1's upload overlaps chunk 0's download.

Device pipeline per batch (unchanged from the fp32 version except I/O):
  phase A  (per 128-point chunk): layernorm -> ffn_dist -> LSH argmax key;
           accumulate per-chunk key histograms; keep z*m / x_dist resident.
  phase A2: counting-sort ranks via prefix sums; indirect-scatter packed rows
           into sorted order (DRAM).
  phase B  (per 128-point bin): pairwise gaussian adjacency + 2 GHConv
           layers; quantize to int8 + row scale; indirect-scatter to the
           original row order.
"""

import threading

import numpy as np

import concourse.bass as bass
import concourse.tile as tile
from concourse import mybir
from concourse.masks import make_identity

dt = mybir.dt
OP = mybir.AluOpType
AF = mybir.ActivationFunctionType
IOA = bass.IndirectOffsetOnAxis

F = 256       # feature dim
D = 128       # distance dim
BIN = 128
OW = F + 4    # int8 output row: 256 data + 4 bytes fp32 row scale

# packed row layout (fp32): [ zm(0:256) | xd(256:384) | m(384) | idx(385) | pad ]
RW = 388
COL_M = 384
COL_IDX = 385


def split_excess_waits(nc):
    """This walrus build rejects instructions carrying more than a couple of
    sem waits (1 for CTRL-class like Drain, ~2 for compute). Move excess
    waits onto extra Drains inserted just before, on the same engine."""
    for f in nc.m.functions:
        for b in f.blocks:
            new_insts = []
            for inst in b.instructions:
                si = getattr(inst, "sync_info", None)
                ow = list(si.on_wait) if si is not None and si.on_wait else []
                limit = 1
                if len(ow) > limit and inst.engine is not None:
                    keep = ow[-limit:]
                    for w in ow[:-limit]:
                        d = mybir.InstNoOp(
                            name=nc.get_next_instruction_name(), ins=[], outs=[]
                        )
                        d.engine = inst.engine
                        d.sync_info = mybir.SyncInfo(on_wait=[w], on_update=[])
                        new_insts.append(d)
                    si.on_wait = keep
                new_insts.append(inst)
            b.instructions = new_insts


def build(nch, KB):
    """Build the Bass module: ONE batch of nch 128-point chunks per core.
    KB = compact-output row bound: unmasked rows scatter to [0, cnt) in
    original order, masked rows (output exactly 0) to dump rows [KB, KB+128)."""
    NP = nch * BIN            # points per batch
    NBINS = nch               # bins == chunks
    CB = NBINS // 2           # used codebook columns
    NK = 2 * NBINS - 1        # distinct sort keys: 0 .. NK-1
    NKP = NK + 1              # padded width
    f32 = dt.float32
    bf16 = dt.bfloat16

    nc = bass.Bass("TRN2", target_bir_lowering=False, debug=False)

    x_in = nc.dram_tensor("x", [NP, F], dt.int8, kind="ExternalInput").ap()
    m_in = nc.dram_tensor("m", [NP, 1], f32, kind="ExternalInput").ap()
    key_in = nc.dram_tensor("keyf", [NP, 1], f32, kind="ExternalInput").ap()
    eps_in = nc.dram_tensor("epsr", [NP, 1], f32, kind="ExternalInput").ap()
    wspec = [
        ("W1g", [F, D]), ("b1gb", [1, D]), ("W2", [D, D]), ("b2", [1, D]),
        ("th0", [F, F]), ("Wh0", [F, F]), ("Wt0", [F, F]),
        ("bth0", [1, F]), ("bhh0", [1, F]), ("bgt0", [1, F]),
        ("th1", [F, F]), ("Wh1", [F, F]), ("Wt1", [F, F]), ("bt1", [1, F]),
    ]
    wdram = {n: nc.dram_tensor(n, s, f32, kind="ExternalInput").ap() for n, s in wspec}
    outq_d = nc.dram_tensor("outq0", [KB + 128, OW], dt.int8,
                            kind="ExternalOutput").ap()
    psort_d = nc.dram_tensor("psort0", [NP, RW], f32, kind="Internal").ap()

    with tile.TileContext(nc) as tc:
        with tc.tile_pool(name="init", bufs=1) as ip:
            ident = ip.tile([128, 128], f32)
            make_identity(nc, ident[:])
            ones_row_f = ip.tile([1, 128], f32)
            nc.vector.memset(ones_row_f[:], 1.0)
            eps_fx = ip.tile([128, 1], f32)
            nc.vector.memset(eps_fx[:], 1e-6)
            iota_p_i = ip.tile([128, 1], dt.int32)
            nc.gpsimd.iota(iota_p_i[:], [[0, 1]], base=0, channel_multiplier=1)
            iota_p_f = ip.tile([128, 1], f32)
            nc.vector.tensor_copy(iota_p_f[:], iota_p_i[:])
            dump_col = ip.tile([128, 1], f32)
            nc.vector.tensor_scalar_add(dump_col[:], iota_p_f[:], float(KB))
            iota_row_i = ip.tile([128, NKP], dt.int32)
            nc.gpsimd.iota(iota_row_i[:], [[1, NKP]], base=0, channel_multiplier=0)
            iota_row_f = ip.tile([128, NKP], f32)
            nc.vector.tensor_copy(iota_row_f[:], iota_row_i[:])
            # strictly-lower mask Tp[p', p] = 1 if p' < p  (for within-chunk cumsum)
            iota_r128_i = ip.tile([128, 128], dt.int32)
            nc.gpsimd.iota(iota_r128_i[:], [[1, 128]], base=0, channel_multiplier=0)
            iota_r128_f = ip.tile([128, 128], f32)
            nc.vector.tensor_copy(iota_r128_f[:], iota_r128_i[:])
            Tp_bf = ip.tile([128, 128], bf16)
            nc.vector.tensor_scalar(
                out=Tp_bf[:], in0=iota_r128_f[:], scalar1=iota_p_f[:],
                scalar2=None, op0=OP.is_gt)
            ones_col_bf = ip.tile([128, 1], bf16)
            nc.vector.memset(ones_col_bf[:], 1.0)

            # weights to SBUF
            wsb = {}
            for n, s in wspec:
                shp = ([128, s[0] // 128, s[1]] if s[0] > 128 else list(s))
                src = (wdram[n].rearrange("(c p) m -> p c m", p=128)
                       if s[0] > 128 else wdram[n][:])
                t = ip.tile(shp, f32, tag=f"w_{n}")
                nc.gpsimd.dma_start(out=t[:], in_=src)
                wsb[n] = t

            _one_batch(tc, nc, nch, NP, NBINS, CB, NK, NKP,
                       x_in, m_in, key_in, eps_in, wsb, outq_d, psort_d,
                       ident, eps_fx, iota_p_f, iota_row_f, Tp_bf,
                       ones_col_bf, ones_row_f, dump_col)

    split_excess_waits(nc)
    return nc


def _one_batch(tc, nc, nch, NP, NBINS, CB, NK, NKP,
               x_in, m_in, key_in, eps_in, wsb, outq_d, psort_d,
               ident, eps_fx, iota_p_f, iota_row_f, Tp_bf,
               ones_col_bf, ones_row_f, dump_col):
    f32 = dt.float32
    bf16 = dt.bfloat16

    with tc.tile_pool(name="res", bufs=1) as rp, \
         tc.tile_pool(name="resps", bufs=1, space="PSUM") as rpp:
        packed = rp.tile([128, nch, RW], f32)     # resident z*m / xd / m / cidx
        key_all = rp.tile([128, nch], f32)
        rank_f = rp.tile([128, nch], f32)
        rank_u = rp.tile([128, nch], dt.uint32)
        T_lo = rpp.tile([NBINS, nch], f32, space="PSUM")
        T_hi = rpp.tile([NBINS - 1, nch], f32, space="PSUM")
        cnt_ps = rpp.tile([1, nch], f32, space="PSUM")

        # ---------------- phase A ----------------
        with tc.tile_pool(name="pa", bufs=3) as pa, \
             tc.tile_pool(name="paps", bufs=1, space="PSUM") as pap:
            for c in range(nch):
                row0 = c * 128
                x_t8 = pa.tile([128, F], dt.int8)
                nc.sync.dma_start(out=x_t8[:], in_=x_in[row0:row0 + 128, :])
                x_t = pa.tile([128, F], f32)
                nc.vector.tensor_copy(x_t[:], x_t8[:])
                nc.sync.dma_start(out=packed[:, c, COL_M:COL_M + 1],
                                  in_=m_in[row0:row0 + 128, :])
                m_ap = packed[:, c, COL_M:COL_M + 1]
                eps_t = pa.tile([128, 1], f32)
                nc.sync.dma_start(out=eps_t[:], in_=eps_in[row0:row0 + 128, :])
                # within-chunk exclusive cumsum of the mask + per-chunk count
                # (for the compact-output row index)
                mcol_bf = pa.tile([128, 1], bf16)
                nc.vector.tensor_copy(mcol_bf[:], m_ap)
                cw_ps = pap.tile([128, 1], f32, space="PSUM")
                nc.tensor.matmul(cw_ps[:], lhsT=Tp_bf[:], rhs=mcol_bf[:],
                                 start=True, stop=True)
                nc.scalar.activation(out=packed[:, c, COL_IDX:COL_IDX + 1],
                                     in_=cw_ps[:], func=AF.Copy)
                nc.tensor.matmul(cnt_ps[0:1, c:c + 1], lhsT=mcol_bf[:],
                                 rhs=ones_col_bf[:], start=True, stop=True)

                st = pa.tile([128, 6], f32)
                nc.vector.bn_stats(out=st[:], in_=x_t[:])
                mv = pa.tile([128, 2], f32)
                nc.vector.bn_aggr(out=mv[:], in_=st[:])
                nc.scalar.activation(out=mv[:, 1:2], in_=mv[:, 1:2],
                                     func=AF.Sqrt, bias=eps_t[:])
                nc.vector.reciprocal(out=mv[:, 1:2], in_=mv[:, 1:2])
                z_t = pa.tile([128, F], f32)
                nc.vector.tensor_scalar(
                    out=z_t[:], in0=x_t[:], scalar1=mv[:, 0:1],
                    scalar2=mv[:, 1:2], op0=OP.subtract, op1=OP.mult)
                # zm into packed (gpsimd: SBUF only)
                nc.gpsimd.tensor_scalar_mul(packed[:, c, 0:F], z_t[:], m_ap)

                # zT (feature-major) for the ffn matmuls
                zT_ps = pap.tile([128, 2, 128], f32, space="PSUM")
                for k in range(2):
                    nc.tensor.transpose(zT_ps[:, k, :],
                                        z_t[:, k * 128:(k + 1) * 128], ident[:])
                zT_sb = pa.tile([128, 2, 128], f32)
                nc.scalar.activation(out=zT_sb[:], in_=zT_ps[:], func=AF.Copy)

                # hT = W1g^T zT + b1gb  (feature-major [D, pts])
                h_ps = pap.tile([128, 128], f32, space="PSUM")
                nc.tensor.matmul(h_ps[:], lhsT=wsb["W1g"][:, 0, :],
                                 rhs=zT_sb[:, 0, :], start=True, stop=False)
                nc.tensor.matmul(h_ps[:], lhsT=wsb["W1g"][:, 1, :],
                                 rhs=zT_sb[:, 1, :], start=False, stop=False)
                nc.tensor.matmul(h_ps[:], lhsT=wsb["b1gb"][:],
                                 rhs=ones_row_f[:], start=False, stop=True)
                # elu
                e_t = pa.tile([128, 128], f32)
                nc.vector.tensor_scalar_min(e_t[:], h_ps[:], 0.0)
                nc.scalar.activation(out=e_t[:], in_=e_t[:], func=AF.Exp)
                r_t = pa.tile([128, 128], f32)
                nc.scalar.activation(out=r_t[:], in_=h_ps[:], func=AF.Relu)
                hTe = pa.tile([128, 128], f32)
                nc.vector.scalar_tensor_tensor(
                    out=hTe[:], in0=e_t[:], scalar=-1.0, in1=r_t[:],
                    op0=OP.add, op1=OP.add)

                # xdT = W2^T hTe + b2
                xdT_ps = pap.tile([128, 128], f32, space="PSUM")
                nc.tensor.matmul(xdT_ps[:], lhsT=wsb["W2"][:], rhs=hTe[:],
                                 start=True, stop=False)
                nc.tensor.matmul(xdT_ps[:], lhsT=wsb["b2"][:],
                                 rhs=ones_row_f[:], start=False, stop=True)
                xdT_sb = pa.tile([128, 128], f32)
                nc.scalar.activation(out=xdT_sb[:], in_=xdT_ps[:], func=AF.Copy)
                # xd point-major into packed
                xd_ps = pap.tile([128, 128], f32, space="PSUM")
                nc.tensor.transpose(xd_ps[:], xdT_sb[:], ident[:])
                nc.vector.tensor_copy(packed[:, c, F:F + 128], xd_ps[:])

                # LSH key: host-computed (bit-exact vs the reference argmax)
                nc.sync.dma_start(out=key_all[:, c:c + 1],
                                  in_=key_in[row0:row0 + 128, :])
                # onehot -> per-chunk histogram columns
                oh = pa.tile([128, NKP], bf16)
                nc.vector.tensor_scalar(
                    out=oh[:], in0=iota_row_f[:, 0:NKP], scalar1=key_all[:, c:c + 1],
                    scalar2=None, op0=OP.is_equal)
                nc.tensor.matmul(T_lo[:, c:c + 1], lhsT=oh[:, 0:NBINS],
                                 rhs=ones_col_bf[:], start=True, stop=True)
                nc.tensor.matmul(T_hi[:, c:c + 1], lhsT=oh[:, NBINS:NK],
                                 rhs=ones_col_bf[:], start=True, stop=True)

        # ---------------- phase A2: ranks + sort scatter ----------------
        with tc.tile_pool(name="pa2", bufs=2) as p2, \
             tc.tile_pool(name="pa2ps", bufs=1, space="PSUM") as p2p:
            # compact-row offsets: exclusive scan of per-chunk mask counts,
            # broadcast to all partitions, added to the within-chunk ranks
            cnt_sb = p2.tile([1, nch], f32)
            nc.scalar.activation(out=cnt_sb[:], in_=cnt_ps[0:1, :], func=AF.Copy)
            cnt_inc = p2.tile([1, nch], f32)
            nc.vector.tensor_tensor_scan(cnt_inc[:], cnt_sb[:], cnt_sb[:], 0.0,
                                         OP.add, OP.bypass)
            cnt_exc = p2.tile([1, nch], f32)
            nc.vector.tensor_sub(cnt_exc[:], cnt_inc[:], cnt_sb[:])
            offs_bc = p2p.tile([128, nch], f32, space="PSUM")
            nc.tensor.matmul(offs_bc[:], lhsT=ones_row_f[:], rhs=cnt_exc[:],
                             start=True, stop=True)
            nc.vector.tensor_tensor(
                out=packed[:, :, COL_IDX], in0=packed[:, :, COL_IDX],
                in1=offs_bc[:], op=OP.add)
            Tl_sb = p2.tile([NBINS, nch], f32)
            nc.scalar.activation(out=Tl_sb[:], in_=T_lo[:], func=AF.Copy)
            Th_sb = p2.tile([NBINS - 1, nch], f32)
            nc.scalar.activation(out=Th_sb[:], in_=T_hi[:], func=AF.Copy)
            # inclusive scan along chunks
            Sl_in = p2.tile([NBINS, nch], f32)
            nc.vector.tensor_tensor_scan(Sl_in[:], Tl_sb[:], Tl_sb[:], 0.0,
                                         OP.add, OP.bypass)
            Sh_in = p2.tile([NBINS - 1, nch], f32)
            nc.vector.tensor_tensor_scan(Sh_in[:], Th_sb[:], Th_sb[:], 0.0,
                                         OP.add, OP.bypass)
            # exclusive
            Sl_ex = p2.tile([NBINS, nch], f32)
            nc.vector.tensor_sub(Sl_ex[:], Sl_in[:], Tl_sb[:])
            Sh_ex = p2.tile([NBINS - 1, nch], f32)
            nc.vector.tensor_sub(Sh_ex[:], Sh_in[:], Th_sb[:])
            # grand totals -> key-offsets (exclusive cumsum over keys)
            grow_ps = p2p.tile([1, NKP], f32, space="PSUM")
            nc.tensor.transpose(grow_ps[:, 0:NBINS], Sl_in[:, nch - 1:nch],
                                ident[0:NBINS, 0:NBINS])
            nc.tensor.transpose(grow_ps[:, NBINS:NK], Sh_in[:, nch - 1:nch],
                                ident[0:NBINS - 1, 0:NBINS - 1])
            grow_sb = p2.tile([1, NKP], f32)
            nc.vector.memset(grow_sb[:], 0.0)
            nc.scalar.activation(out=grow_sb[:, 0:NK], in_=grow_ps[:, 0:NK],
                                 func=AF.Copy)
            ginc = p2.tile([1, NKP], f32)
            nc.vector.tensor_tensor_scan(ginc[:], grow_sb[:], grow_sb[:], 0.0,
                                         OP.add, OP.bypass)
            gexc = p2.tile([1, NKP], f32)
            nc.vector.tensor_sub(gexc[:], ginc[:], grow_sb[:])
            offs_ps = p2p.tile([NBINS, 2], f32, space="PSUM")
            nc.tensor.transpose(offs_ps[:, 0:1], gexc[:, 0:NBINS], ident[0:1, 0:1])
            nc.tensor.transpose(offs_ps[0:NBINS - 1, 1:2], gexc[:, NBINS:NK],
                                ident[0:1, 0:1])
            offs_sb = p2.tile([NBINS, 2], f32)
            nc.scalar.activation(out=offs_sb[:], in_=offs_ps[:], func=AF.Copy)
            nc.vector.tensor_scalar_add(Sl_ex[:], Sl_ex[:], offs_sb[:, 0:1])
            nc.vector.tensor_scalar_add(Sh_ex[:], Sh_ex[:],
                                        offs_sb[0:NBINS - 1, 1:2])
            # St[c, k] = base for chunk c / key k
            St_ps = p2p.tile([nch, NKP], f32, space="PSUM")
            nc.tensor.transpose(St_ps[:, 0:NBINS], Sl_ex[:],
                                ident[0:NBINS, 0:NBINS])
            nc.tensor.transpose(St_ps[:, NBINS:NK], Sh_ex[:],
                                ident[0:NBINS - 1, 0:NBINS - 1])
            St_sb = p2.tile([nch, NKP], f32)
            nc.vector.memset(St_sb[:], 0.0)
            nc.scalar.activation(out=St_sb[:, 0:NK], in_=St_ps[:, 0:NK],
                                 func=AF.Copy)

            for c in range(nch):
                oh2 = p2.tile([128, NKP], bf16)
                nc.vector.tensor_scalar(
                    out=oh2[:], in0=iota_row_f[:, 0:NKP],
                    scalar1=key_all[:, c:c + 1], scalar2=None, op0=OP.is_equal)
                St_row = p2.tile([1, NKP], f32)
                nc.sync.dma_start(out=St_row[:], in_=St_sb[c:c + 1, :])
                C_ps = p2p.tile([128, NKP], f32, space="PSUM")
                nc.tensor.matmul(C_ps[:], lhsT=Tp_bf[:], rhs=oh2[:],
                                 start=True, stop=False)
                nc.tensor.matmul(C_ps[:], lhsT=ones_row_f[:],
                                 rhs=St_row[:], start=False, stop=True)
                scr = p2.tile([128, NKP], f32)
                nc.vector.tensor_tensor(out=scr[:], in0=oh2[:], in1=C_ps[:],
                                        op=OP.mult)
                nc.vector.tensor_reduce(
                    out=rank_f[:, c:c + 1], in_=scr[:],
                    axis=mybir.AxisListType.X, op=OP.add)
            nc.vector.tensor_copy(rank_u[:], rank_f[:])
            for c in range(nch):
                nc.gpsimd.indirect_dma_start(
                    out=psort_d[:],
                    out_offset=IOA(ap=rank_u[:, c:c + 1], axis=0),
                    in_=packed[:, c, :], in_offset=None)

    # ---------------- phase B: adjacency + GHConv per bin ----------------
    with tc.tile_pool(name="pb", bufs=4) as pb, \
         tc.tile_pool(name="pbps", bufs=1, space="PSUM") as pbp:
        for s in range(NBINS):
            pk = pb.tile([128, RW], f32)
            nc.sync.dma_start(out=pk[:], in_=psort_d[s * 128:(s + 1) * 128, :])
            m_ap = pk[:, COL_M:COL_M + 1]
            # V cols: [na, one, one, na, m]; transposed pair/row tiles all
            # land at partition base 0 (matmul requires equal bases).
            V = pb.tile([128, 5], f32)
            sq = pb.tile([128, 128], f32)
            nc.scalar.activation(out=sq[:], in_=pk[:, F:F + 128],
                                 func=AF.Square, accum_out=V[:, 0:1])
            nc.gpsimd.memset(V[:, 1:3], 1.0)
            nc.gpsimd.tensor_copy(V[:, 3:4], V[:, 0:1])
            nc.gpsimd.tensor_copy(V[:, 4:5], m_ap)
            vt_ps = pbp.tile([2, 384], f32, space="PSUM")
            nc.tensor.transpose(vt_ps[0:2, 0:128], V[:, 0:2], ident[:])
            VTa = pb.tile([2, 128], f32)
            nc.scalar.activation(out=VTa[:], in_=vt_ps[0:2, 0:128],
                                 func=AF.Copy)
            nc.tensor.transpose(vt_ps[0:2, 128:256], V[:, 2:4], ident[:])
            VTb = pb.tile([2, 128], f32)
            nc.scalar.activation(out=VTb[:], in_=vt_ps[0:2, 128:256],
                                 func=AF.Copy)
            nc.tensor.transpose(vt_ps[0:1, 256:384], V[:, 4:5], ident[:])
            mT_sb = pb.tile([1, 128], f32)
            nc.scalar.activation(out=mT_sb[:], in_=vt_ps[0:1, 256:384],
                                 func=AF.Copy)
            # d2 = na_i - 2 xd xd^T + na_j ; M2 = m_i m_j
            adj_ps = pbp.tile([128, 384], f32, space="PSUM")
            xdT_ps = adj_ps[:, 0:128]
            d2_ps = adj_ps[:, 128:256]
            M2_ps = adj_ps[:, 256:384]
            nc.tensor.transpose(xdT_ps, pk[:, F:F + 128], ident[:])
            xdT = pb.tile([128, 128], f32)
            nc.scalar.activation(out=xdT[:], in_=xdT_ps, func=AF.Copy)
            xdTm2 = pb.tile([128, 128], f32)
            nc.scalar.activation(out=xdTm2[:], in_=xdT_ps, func=AF.Copy,
                                 scale=-2.0)
            nc.tensor.matmul(d2_ps, lhsT=xdTm2[:], rhs=xdT[:],
                             start=True, stop=False)
            nc.tensor.matmul(d2_ps, lhsT=VTa[:], rhs=VTb[:],
                             start=False, stop=True)
            nc.tensor.matmul(M2_ps, lhsT=mT_sb[:], rhs=mT_sb[:],
                             start=True, stop=True)
            dsc = pb.tile([128, 128], f32)
            nc.vector.tensor_scalar_max(dsc[:], d2_ps[:], 1e-6)
            nc.scalar.activation(out=dsc[:], in_=dsc[:], func=AF.Sqrt)
            nc.scalar.activation(out=dsc[:], in_=dsc[:], func=AF.Exp,
                                 scale=-0.1)
            dm = pb.tile([128, 128], f32)
            ind = pb.tile([128, 1], f32)
            nc.vector.scalar_tensor_tensor(
                out=dm[:], in0=dsc[:], scalar=1.0, in1=M2_ps[:],
                op0=OP.mult, op1=OP.mult, accum_out=ind[:])
            nrm = pb.tile([128, 1], f32)
            nc.scalar.activation(out=nrm[:], in_=ind[:], func=AF.Sqrt,
                                 bias=eps_fx[:])
            nc.vector.reciprocal(nrm[:], nrm[:])
            nc.vector.tensor_mul(nrm[:], nrm[:], m_ap)

            xb_ap = pk[:, 0:F]
            for li in range(2):
                sfx = "0" if li == 0 else "1"
                mm1 = pbp.tile([128, 512], f32, space="PSUM")
                mm2 = pbp.tile([128, 512], f32, space="PSUM")
                gat_ps = pbp.tile([128, F], f32, space="PSUM")
                xmT_ps = mm1[:, 0:256]
                hom2_ps = mm1[:, 256:512]
                hom_ps = mm2[:, 0:256]
                het_ps = mm2[:, 256:512]
                for k in range(2):
                    nc.tensor.transpose(
                        xmT_ps.rearrange("p (c q) -> p c q", q=128)[:, k, :],
                        xb_ap[:, k * 128:(k + 1) * 128], ident[:])
                xmT = pb.tile([128, 2, 128], f32)
                nc.scalar.activation(out=xmT[:], in_=xmT_ps, func=AF.Copy)
                mT = mT_sb[:]
                # keep each PSUM accumulation group's matmuls consecutive
                for dst, wn, bias in (
                    (hom_ps, "th" + sfx, "bth0" if li == 0 else None),
                    (het_ps, "Wh" + sfx, "bhh0" if li == 0 else None),
                    (gat_ps[:], "Wt" + sfx,
                     "bgt0" if li == 0 else "bt1"),
                ):
                    for k in range(2):
                        nc.tensor.matmul(
                            dst, lhsT=xmT[:, k, :], rhs=wsb[wn][:, k, :],
                            start=(k == 0), stop=(k == 1 and bias is None))
                    if bias is not None:
                        blhs = mT if li == 0 else ones_row_f[:]
                        nc.tensor.matmul(dst, lhsT=blhs, rhs=wsb[bias][:],
                                         start=False, stop=True)
                fh1 = pb.tile([128, F], f32)
                nc.vector.tensor_scalar_mul(fh1[:], hom_ps[:], nrm[:])
                nc.tensor.matmul(hom2_ps[:], lhsT=dm[:], rhs=fh1[:],
                                 start=True, stop=True)
                gate = pb.tile([128, F], f32)
                nc.scalar.activation(out=gate[:], in_=gat_ps[:], func=AF.Sigmoid)
                fh2 = pb.tile([128, F], f32)
                nc.vector.tensor_scalar_mul(fh2[:], hom2_ps[:], nrm[:])
                nc.vector.tensor_sub(fh2[:], fh2[:], het_ps[:])
                nc.vector.tensor_mul(gate[:], gate[:], fh2[:])
                nc.vector.tensor_add(fh2[:], gate[:], het_ps[:])  # pre-act
                emin = pb.tile([128, F], f32)
                nc.gpsimd.tensor_scalar_min(emin[:], fh2[:], 0.0)
                nc.scalar.activation(out=emin[:], in_=emin[:], func=AF.Exp)
                er = pb.tile([128, F], f32)
                nc.scalar.activation(out=er[:], in_=fh2[:], func=AF.Relu)
                nc.vector.scalar_tensor_tensor(
                    out=emin[:], in0=emin[:], scalar=-1.0, in1=er[:],
                    op0=OP.add, op1=OP.add)
                out_t = pb.tile([128, F], f32)
                nc.gpsimd.tensor_scalar_mul(out_t[:], emin[:], m_ap)
                xb_ap = out_t[:]

            # int8 quantization with per-row scale, packed into one row
            absf = pb.tile([128, F], f32)
            nc.scalar.activation(out=absf[:], in_=xb_ap, func=AF.Abs)
            rmax = pb.tile([128, 1], f32)
            nc.vector.tensor_reduce(out=rmax[:], in_=absf[:],
                                    axis=mybir.AxisListType.X, op=OP.max)
            nc.vector.tensor_scalar_max(rmax[:], rmax[:], 1e-30)
            inv = pb.tile([128, 1], f32)
            nc.vector.reciprocal(inv[:], rmax[:])
            scrow = pb.tile([128, 1], f32)
            nc.vector.tensor_scalar_mul(scrow[:], rmax[:], 1.0 / 127.0)
            qf = pb.tile([128, F], f32)
            nc.vector.tensor_scalar_mul(qf[:], xb_ap, inv[:])
            nc.vector.tensor_scalar(out=qf[:], in0=qf[:], scalar1=127.0,
                                    scalar2=127.0, op0=OP.mult, op1=OP.min)
            q8 = pb.tile([128, OW], dt.int8)
            nc.vector.tensor_copy(q8[:, 0:F], qf[:])
            nc.vector.tensor_copy(q8[:, F:OW], scrow[:].bitcast(dt.int8))

            cidf = pb.tile([128, 1], f32)
            nc.vector.tensor_sub(cidf[:], pk[:, COL_IDX:COL_IDX + 1],
                                 dump_col[:])
            nc.vector.tensor_scalar_mul(cidf[:], cidf[:], m_ap)
            nc.vector.tensor_add(cidf[:], cidf[:], dump_col[:])
            idx_u = pb.tile([128, 1], dt.uint32)
            nc.vector.tensor_copy(idx_u[:], cidf[:])
            nc.gpsimd.indirect_dma_start(
                out=outq_d[:], out_offset=IOA(ap=idx_u[:, 0:1], axis=0),
                in_=q8[:], in_offset=None)


def _fold_weights(inputs):
    g = inputs["ln_gamma"].astype(np.float32)
    be = inputs["ln_beta"].astype(np.float32)
    W1 = inputs["W1"].astype(np.float32)
    b1 = inputs["b1"].astype(np.float32)
    w = {
        "W1g": g[:, None] * W1,
        "b1gb": (b1 + be @ W1)[None, :],
        "W2": inputs["W2"].astype(np.float32),
        "b2": inputs["b2"].astype(np.float32)[None, :],
        "th1": inputs["th1"].astype(np.float32),
        "Wh1": inputs["Wh1"].astype(np.float32),
        "Wt1": inputs["Wt1"].astype(np.float32),
        "bt1": inputs["bt1"].astype(np.float32)[None, :],
    }
    for nm in ("th0", "Wh0", "Wt0"):
        w[nm] = g[:, None] * inputs[nm].astype(np.float32)
    w["bth0"] = (be @ inputs["th0"].astype(np.float32))[None, :]
    w["bhh0"] = (be @ inputs["Wh0"].astype(np.float32))[None, :]
    w["bgt0"] = (inputs["bt0"].astype(np.float32) +
                 be @ inputs["Wt0"].astype(np.float32))[None, :]
    return {k: np.ascontiguousarray(v, dtype=np.float32) for k, v in w.items()}


_BUILD_CACHE = {}


def _get_nc(nch, KB):
    if (nch, KB) not in _BUILD_CACHE:
        _BUILD_CACHE[(nch, KB)] = build(nch, KB)
    return _BUILD_CACHE[(nch, KB)]


_RUNNER_CACHE = {}


def _get_runner(nch, n_cores, KB):
    """Cached jitted SPMD executor (re-jitting per call costs seconds)."""
    key = (nch, n_cores, KB)
    if key in _RUNNER_CACHE:
        return _RUNNER_CACHE[key]
    import jax
    from jax.sharding import Mesh, PartitionSpec, NamedSharding
    from jax.experimental.shard_map import shard_map
    from concourse import bass2jax

    bass2jax.install_neuronx_cc_hook()
    nc = _get_nc(nch, KB)
    partition_name = (nc.partition_id_tensor.name
                      if nc.partition_id_tensor else None)
    in_names, out_names, out_avals, zero_shapes = [], [], [], []
    for alloc in nc.m.functions[0].allocations:
        if not isinstance(alloc, mybir.MemoryLocationSet):
            continue
        name = alloc.memorylocations[0].name
        if alloc.kind == "ExternalInput":
            if name != partition_name:
                in_names.append(name)
        elif alloc.kind == "ExternalOutput":
            out_names.append(name)
            shape = tuple(alloc.tensor_shape)
            dtype = mybir.dt.np(alloc.dtype)
            out_avals.append(jax.core.ShapedArray(shape, dtype))
            zero_shapes.append((shape, dtype))
    n_params = len(in_names)
    all_names = in_names + out_names
    if partition_name is not None:
        all_names = all_names + [partition_name]

    def _body(*args):
        operands = list(args)
        if partition_name is not None:
            operands.append(bass2jax.partition_id_tensor())
        outs = bass2jax._bass_exec_p.bind(
            *operands,
            out_avals=tuple(out_avals),
            in_names=tuple(all_names),
            out_names=tuple(out_names),
            lowering_input_output_aliases=(),
            sim_require_finite=True,
            sim_require_nnan=True,
            nc=nc,
        )
        return tuple(outs)

    devices = jax.devices()[:n_cores]
    mesh = Mesh(np.asarray(devices), ("core",))
    in_specs = (PartitionSpec("core"),) * (n_params + len(out_names))
    out_specs = (PartitionSpec("core"),) * len(out_names)
    sharded = jax.jit(
        shard_map(_body, mesh=mesh, in_specs=in_specs, out_specs=out_specs,
                  check_rep=False),
        keep_unused=True)
    # zero output buffers staged on device ONCE and reused read-only
    shard = NamedSharding(mesh, PartitionSpec("core"))
    dev_zeros = [
        jax.device_put(np.zeros((n_cores * s0[0], *s0[1:]), d), shard)
        for s0, d in zero_shapes]
    runner = (sharded, in_names, out_names, out_avals, dev_zeros, shard)
    _RUNNER_CACHE[key] = runner
    return runner


_WCACHE = {}


def _dev_weights(inputs, n_cores, shard):
    """Folded weights, tiled per-core and staged on device once (cached by
    content hash -- ~5.6MB of wire saved per call)."""
    import hashlib
    import jax

    h = hashlib.blake2b(digest_size=16)
    for kk in ("ln_gamma", "ln_beta", "W1", "b1", "W2", "b2", "th0", "Wh0",
               "Wt0", "bt0", "th1", "Wh1", "Wt1", "bt1"):
        h.update(np.ascontiguousarray(inputs[kk], dtype=np.float32).tobytes())
    dig = h.hexdigest()
    if dig in _WCACHE:
        return _WCACHE[dig]
    w = _fold_weights(inputs)
    gw = {n: jax.device_put(np.concatenate([v] * n_cores, axis=0), shard)
          for n, v in w.items()}
    jax.block_until_ready(list(gw.values()))
    _WCACHE[dig] = gw
    return gw


_JITS = {}


def _get_jits(nbins):
    """Two cached jax-CPU jits:
    keys_of -- the LSH sort keys, with EXACTLY the op sequence of the
      reference (bit-identical argmax; one flipped bin costs ~1e-1 max-rel).
    pack_of -- row-centered int8 quantization of x + the per-row rescaled
      LN epsilon (LayerNorm is invariant to per-row shift/scale)."""
    if nbins in _JITS:
        return _JITS[nbins]
    import jax
    import jax.numpy as jnp

    @jax.jit
    def keys_of(xv, mskv, ln_gamma, ln_beta, W1, b1, W2, b2, codebook):
        mu = jnp.mean(xv, axis=-1, keepdims=True)
        var = jnp.mean(jnp.square(xv - mu), axis=-1, keepdims=True)
        xn = (xv - mu) * jax.lax.rsqrt(var + 1e-6) * ln_gamma + ln_beta
        x_dist = jax.nn.elu(xn @ W1 + b1) @ W2 + b2
        mul = x_dist @ codebook
        cmul = jnp.concatenate([mul, -mul], axis=-1)
        key = jnp.argmax(cmul, axis=-1) + jnp.where(~mskv, nbins - 1, 0)
        return key.astype(jnp.float32)

    @jax.jit
    def pack_of(xv):
        mu = jnp.mean(xv, axis=-1, keepdims=True)
        xc = xv - mu
        rm = jnp.maximum(jnp.max(jnp.abs(xc), axis=-1, keepdims=True), 1e-30)
        c = 127.0 / rm
        q8 = jnp.rint(xc * c).astype(jnp.int8)
        return q8, (1e-6 * c * c).astype(jnp.float32)

    _JITS[nbins] = (keys_of, pack_of)
    return _JITS[nbins]


def run(inputs, nb=2, nch=100, n_cores=8, ghconv_dtype=None, trace=False):
    """inputs: dict with x [B, NP, F] float32, msk [B, NP] bool + weights.
    B must equal n_cores * nb; processed as nb pipelined chunks of one batch
    per core."""
    import jax
    from concurrent.futures import ThreadPoolExecutor

    NP = nch * BIN
    x = np.asarray(inputs["x"])
    msk = np.asarray(inputs["msk"])
    B = x.shape[0]
    assert B == n_cores * nb
    cnts = msk.reshape(B, NP).sum(axis=1).astype(np.int64)
    KB = 56 * BIN                 # 0.56*NP; ~13 sigma above a fair-coin mask
    if int(cnts.max()) > KB:
        KB = NP                   # pathological mask density: no compaction
    sharded, in_names, out_names, out_avals, dev_zeros, shard = _get_runner(
        nch, n_cores, KB)
    keys_of, pack_of = _get_jits(nch)
    gw = _dev_weights(inputs, n_cores, shard)
    kargs = (inputs["ln_gamma"], inputs["ln_beta"], inputs["W1"],
             inputs["b1"], inputs["W2"], inputs["b2"],
             np.ascontiguousarray(inputs["codebook"][:, :nch // 2]))

    cpu = jax.devices("cpu")[0]
    futs = []
    with jax.default_device(cpu):
        for k in range(nb):
            xs = x[k * n_cores:(k + 1) * n_cores]
            ms = msk[k * n_cores:(k + 1) * n_cores]
            # pack + start the async x upload FIRST, so the wire streams
            # while the (CPU-bound) exact key computation runs
            q8, epsr = pack_of(xs)
            xq = np.asarray(q8).reshape(n_cores * NP, F)
            dxq = jax.device_put(xq, shard)
            epsr = np.asarray(epsr).reshape(n_cores * NP, 1)
            keys = np.asarray(keys_of(xs, ms, *kargs)).reshape(-1, 1)
            mf = np.ascontiguousarray(
                ms.reshape(n_cores * NP, 1), dtype=np.float32)
            args = [dxq if n == "x" else mf if n == "m" else
                    keys if n == "keyf" else epsr if n == "epsr" else gw[n]
                    for n in in_names]
            futs.append(sharded(*args, *dev_zeros))

    out = np.zeros((B, NP, F), np.float32)   # masked rows are exactly 0

    def fetch_one(k, s_):
        core = s_.index[0].start // (KB + 128) if s_.index[0].start else 0
        b = k * n_cores + core
        cnt = int(cnts[b])
        a = np.asarray(s_.data)              # [KB+128, 260] int8
        sc = np.ascontiguousarray(a[:cnt, F:OW]).view(np.float32)
        out[b][msk[b]] = a[:cnt, :F].astype(np.float32) * sc

    with ThreadPoolExecutor(max_workers=8) as ex:
        jobs = []
        for k in range(nb):
            shards = list(futs[k][0].addressable_shards)
            for s_ in shards:
                s_.data.copy_to_host_async()
            jobs += [ex.submit(fetch_one, k, s_) for s_ in shards]
        for j in jobs:
            j.result()
    return out, None


def kernel(**inputs):
    out, _ = run(inputs, nb=2, nch=100, n_cores=8)
    return out
